# revision 21
# baseline (speedup 1.0000x reference)
"""NT-Xent loss on 8 Trainium2 NeuronCores (Bass/Tile).

Math
----
reference: rows = interleave(zjs, zis) [2B, D]; zn = rows/max(|row|,eps);
S = (zn @ zn.T)/0.5; mask diag; loss = -mean_i log_softmax(S)[i, pair(i)].

The loss is invariant to any joint row/column permutation, so we use the
STACKED order rows = [zjs; zis] with pair(i) = i +- B.  Since every score
is 2*cos <= 2 and the diagonal exp(2*cos_ii - 2) == 1 (+/- fp eps), no
masking or row-max pass is needed:

    lse_i  = 2 + ln( sum_j exp(2 cos_ij - 2) - 1 )
    loss   = 2 + ( sum_i ln(rowsum_i - 1) - 2 * sum_i cos_{i,pair(i)} ) / 2B

Distribution: each core gets the full transposed rep matrix [D, 2B]
(= "all-gathered Zn" state), ROLLED along columns by c*1024 so that the
uniform SPMD program always treats columns [0:1024] as its local row
block and [4096:5120] as the positive partners.  Each core normalizes
the full matrix (cheap), computes its 1024x8192 block of the similarity
matrix in bf16 on the TensorEngine, fuses exp+rowsum on the Scalar
engine (accum_out), and emits two partial sums; the host combines 8
pairs of scalars.

Scheduling: engines execute instructions in program order, so emission
hand-pipelines per-engine streams (normalize chunk c interleaved with
main-phase columns of earlier chunks).  Inverse norms are computed as
Exp(-0.5*Ln(ss)); with the ACT table set pinned to
natural_log_exp_and_others, the kernel performs zero table reloads.
A short warm-up burst of matmuls lifts the PE HAM clock-gate to 8/8
before the first real matmul.

Host-side work is layout-only (concat/transpose/roll/replicate): all
arithmetic (normalization, matmul, softmax, log, reductions) is on
device.
"""

import numpy as np
from contextlib import ExitStack

import concourse.bass as bass
import concourse.tile as tile
from concourse import bacc, mybir
from concourse.bass_utils import run_bass_kernel_spmd
from concourse._compat import with_exitstack

B = 4096
D = 256
N = 2 * B                 # 8192 rows/cols of the similarity matrix
N_CORES = 8
LOCAL = N // N_CORES      # 1024 rows per core
CHUNK = 2048              # normalize / matmul-group column chunk
NCHUNK = N // CHUNK       # 4
KC = D // 128             # 2 contraction chunks of 128
MTILES = LOCAL // 128     # 8 m-tiles of 128 rows
F32 = mybir.dt.float32
BF16 = mybir.dt.bfloat16
AF = mybir.ActivationFunctionType
X = mybir.AxisListType.X

@with_exitstack
def _ntxent_kernel(ctx: ExitStack, tc: tile.TileContext, rt_ap, out_ap):
    nc = tc.nc

    sb_rt = ctx.enter_context(tc.tile_pool(name="rt", bufs=KC * NCHUNK))
    sb_sq = ctx.enter_context(tc.tile_pool(name="sq", bufs=4))
    sb_inv = ctx.enter_context(tc.tile_pool(name="inv", bufs=4))
    sb_znt = ctx.enter_context(tc.tile_pool(name="znt", bufs=1))
    sb_dmy = ctx.enter_context(tc.tile_pool(name="dmy", bufs=2))
    sb_fin = ctx.enter_context(tc.tile_pool(name="fin", bufs=1))
    ps = ctx.enter_context(tc.tile_pool(name="ps", bufs=2, space="PSUM"))

    # constants
    ones128 = sb_fin.tile([128, 1], F32, tag="ones128")
    nc.vector.memset(ones128[:], 1.0)
    onesb = sb_fin.tile([128, 128], BF16, tag="onesb")
    nc.vector.memset(onesb[:], 1.0)
    neg2 = sb_fin.tile([128, 1], F32, tag="neg2")
    nc.vector.memset(neg2[:], -2.0)
    neg1 = sb_fin.tile([128, 1], F32, tag="neg1")
    nc.vector.memset(neg1[:], -1.0)

    # persistent tiles
    znt = sb_znt.tile([128, KC, N], BF16)            # normalized, transposed reps
    racc = sb_fin.tile([128, MTILES * NCHUNK], F32, tag="racc")
    lgacc = sb_fin.tile([128, MTILES], F32, tag="lgacc")
    pacc = sb_fin.tile([128, KC], F32, tag="pacc")

    # all input DMAs issued up-front
    rtk = {}
    for c in range(NCHUNK):
        for k in range(KC):
            t = sb_rt.tile([128, CHUNK], F32, tag="rt")
            nc.sync.dma_start(out=t[:], in_=rt_ap[k][:, bass.ds(c * CHUNK, CHUNK)])
            rtk[(c, k)] = t

    def squares(c):
        sqs = []
        for k in range(KC):
            sq = sb_sq.tile([128, CHUNK], BF16, tag="sq")
            nc.vector.tensor_mul(sq[:], rtk[(c, k)][:], rtk[(c, k)][:])
            sqs.append(sq)
        return sqs

    def colsum(c, sqs):
        # column sums of squares, broadcast across partitions via the
        # ones[128,128] stationary operand
        ns2b = ps.tile([128, CHUNK], F32, tag="ps")
        for k in range(KC):
            for j in range(CHUNK // 512):
                jsl = bass.ds(j * 512, 512)
                nc.tensor.matmul(ns2b[:, jsl], onesb[:], sqs[k][:, jsl],
                                 start=(k == 0), stop=(k == KC - 1))
        return ns2b

    def inv_ln(c, ns2b):
        # first half of 1/sqrt(ss) = exp(-0.5*ln(ss)); both functions
        # live in the pinned natural_log_exp table set (no reloads)
        lnb = sb_inv.tile([128, CHUNK], F32, tag="inv")
        nc.scalar.activation(lnb[:], ns2b[:], AF.Ln, bias=0.0, scale=1.0)
        return lnb

    def inv_exp(c, lnb):
        invb = sb_inv.tile([128, CHUNK], F32, tag="inv")
        nc.scalar.activation(invb[:], lnb[:], AF.Exp, bias=0.0, scale=-0.5)
        return invb

    def apply(c, invb):
        csl = bass.ds(c * CHUNK, CHUNK)
        for k in range(KC):
            nc.vector.tensor_mul(znt[:, k, csl], rtk[(c, k)][:], invb[:])

    def main_column(g, ms):
        # 128x2048 similarity blocks for m-tiles `ms` against chunk g
        for m in ms:
            msl = bass.ds(m * 128, 128)
            pst = ps.tile([128, CHUNK], F32, tag="ps")
            for k in range(KC):
                for j in range(CHUNK // 512):
                    jsl = bass.ds(j * 512, 512)
                    nc.tensor.matmul(pst[:, jsl], znt[:, k, msl],
                                     znt[:, k, bass.ds(g * CHUNK + j * 512, 512)],
                                     start=(k == 0), stop=(k == KC - 1))
            dmy = sb_dmy.tile([128, CHUNK], BF16, tag="dmy")
            nc.scalar.activation(dmy[:], pst[:], AF.Exp, bias=neg2[:], scale=2.0,
                                 accum_out=racc[:, bass.ds(m * NCHUNK + g, 1)])

    def pos_phase():
        # positive-pair cosines for local rows (cols 0:1024 x 4096:5120)
        for k in range(KC):
            pprod = sb_dmy.tile([128, LOCAL], BF16, tag="dmy")
            nc.vector.tensor_mul(pprod[:], znt[:, k, 0:LOCAL],
                                 znt[:, k, bass.ds(B, LOCAL)])
            nc.vector.reduce_sum(pacc[:, bass.ds(k, 1)], pprod[:], axis=X)

    # Pin the ACT table set to natural_log_exp_and_others (id 6): it
    # contains every function this kernel uses (Ln, Exp, Copy), so
    # walrus's lower_act inserts no further table loads.  Without this,
    # Ln and Exp resolve to two different sets and every invnorm pair
    # costs two ~1.3us reloads.
    nc.scalar.add_instruction(mybir.InstLoadActFuncSet(
        name=nc.get_next_instruction_name(), ins=[], outs=[],
        act_func_set_id=6))

    # PE warm-up: ~6us of throwaway matmuls so the HAM clock-gate is at
    # 8/8 before the first real matmul (the PE starts cold at 1.2 GHz;
    # the window needs ~3.4us of sustained activity).  Reads the first
    # input chunk so it starts as soon as that DMA lands.
    wp = ps.tile([1, 512], F32, tag="ps")
    for w in range(8):
        nc.tensor.matmul(wp[:], ones128[:], rtk[(0, 0)][:, bass.ds(0, 512)],
                         start=True, stop=True)

    # Hand-pipelined emission. Engines execute their streams in program
    # order, so each engine's stream must avoid waiting on work that a
    # LATER entry of another engine's stream produces.
    #   DVE: sq0 sq1 app0 sq2 app1 sq3 app2 pos app3
    #   PE : warm cs0 cs1 g0[0:4] cs2 g0[4:] g1[0:4] cs3 g1[4:] g2 g3
    #   ACT: inv0 inv1 e(g0,0:4) inv2 e(g0,4:) e(g1,0:4) inv3 e(g1,4:) ...
    front = list(range(MTILES // 2))
    back = list(range(MTILES // 2, MTILES))
    sq0 = squares(0)
    sq1 = squares(1)
    ns0 = colsum(0, sq0)
    ns1 = colsum(1, sq1)
    ln0 = inv_ln(0, ns0)
    ln1 = inv_ln(1, ns1)
    iv0 = inv_exp(0, ln0)
    iv1 = inv_exp(1, ln1)
    apply(0, iv0)
    apply(1, iv1)
    sq2 = squares(2)
    sq3 = squares(3)
    main_column(0, front)
    ns2 = colsum(2, sq2)
    ln2 = inv_ln(2, ns2)
    main_column(0, back)
    iv2 = inv_exp(2, ln2)
    apply(2, iv2)
    main_column(1, front)
    ns3 = colsum(3, sq3)
    ln3 = inv_ln(3, ns3)
    main_column(1, back)
    iv3 = inv_exp(3, ln3)
    apply(3, iv3)
    pos_phase()
    main_column(2, range(MTILES))
    main_column(3, range(MTILES))

    # rowsums -> ln(rowsum - 1) in one op per phase
    rsall = sb_fin.tile([128, MTILES], F32, tag="rsall")
    nc.vector.reduce_sum(
        rsall[:], racc[:].rearrange("p (m g) -> p m g", g=NCHUNK), axis=X)
    nc.scalar.activation(lgacc[:], rsall[:], AF.Ln, bias=neg1[:], scale=1.0)

    # ---- fold to two scalars ----
    fin = sb_fin.tile([128, 2], F32, tag="fin")
    nc.vector.reduce_sum(fin[:, 0:1], lgacc[:], axis=X)
    nc.vector.reduce_sum(fin[:, 1:2], pacc[:], axis=X)
    psf = ps.tile([1, 2], F32, tag="ps")
    nc.tensor.matmul(psf[:], ones128[:], fin[:], start=True, stop=True)
    ob = sb_fin.tile([1, 2], F32, tag="ob")
    nc.scalar.copy(ob[:], psf[:])
    nc.sync.dma_start(out=out_ap[:, :], in_=ob[:])


_NC_CACHE = None


def _build_program():
    global _NC_CACHE
    if _NC_CACHE is not None:
        return _NC_CACHE
    nc = bacc.Bacc("TRN2", target_bir_lowering=False, debug=False,
                   num_devices=N_CORES)
    rt = nc.dram_tensor("rt", [KC, 128, N], F32, kind="ExternalInput").ap()
    out = nc.dram_tensor("out", [1, 2], F32, kind="ExternalOutput").ap()
    with tile.TileContext(nc) as tc:
        _ntxent_kernel(tc, rt, out)
    nc.finalize()
    _NC_CACHE = nc
    return nc


def kernel(zis: np.ndarray, zjs: np.ndarray) -> np.ndarray:
    assert zis.shape == (B, D) and zjs.shape == (B, D)
    nc = _build_program()

    # Host prep (layout only): stack, transpose to [D, N], split the
    # contraction dim, and roll columns so each core's local block is
    # at a uniform offset.
    rt_full = np.ascontiguousarray(
        np.concatenate([zjs, zis], axis=0).T.astype(np.float32, copy=False)
    ).reshape(KC, 128, N)

    in_maps = []
    for c in range(N_CORES):
        rolled = np.roll(rt_full, -c * LOCAL, axis=2)
        in_maps.append({"rt": np.ascontiguousarray(rolled)})

    res = run_bass_kernel_spmd(nc, in_maps, core_ids=list(range(N_CORES)))

    log_sum = 0.0
    pos_sum = 0.0
    for c in range(N_CORES):
        o = res.results[c]["out"]
        log_sum += float(o[0, 0])
        pos_sum += float(o[0, 1])
    loss = 2.0 + (log_sum - 2.0 * pos_sum) / N
    return np.asarray(loss, dtype=np.float32)


# revision 22
# speedup vs baseline: 1.0139x; 1.0139x over previous
"""NT-Xent loss on 8 Trainium2 NeuronCores (Bass/Tile).

Math
----
reference: rows = interleave(zjs, zis) [2B, D]; zn = rows/max(|row|,eps);
S = (zn @ zn.T)/0.5; mask diag; loss = -mean_i log_softmax(S)[i, pair(i)].

The loss is invariant to any joint row/column permutation, so we use the
STACKED order rows = [zjs; zis] with pair(i) = i +- B.  Since every score
is 2*cos <= 2 and the diagonal exp(2*cos_ii - 2) == 1 (+/- fp eps), no
masking or row-max pass is needed:

    lse_i  = 2 + ln( sum_j exp(2 cos_ij - 2) - 1 )
    loss   = 2 + ( sum_i ln(rowsum_i - 1) - 2 * sum_i cos_{i,pair(i)} ) / 2B

Distribution: each core gets the full transposed rep matrix [D, 2B]
(= "all-gathered Zn" state), ROLLED along columns by c*1024 so that the
uniform SPMD program always treats columns [0:1024] as its local row
block and [4096:5120] as the positive partners.  Each core normalizes
the full matrix (cheap), computes its 1024x8192 block of the similarity
matrix in bf16 on the TensorEngine, fuses exp+rowsum on the Scalar
engine (accum_out), and emits two partial sums; the host combines 8
pairs of scalars.

Scheduling: engines execute instructions in program order, so emission
hand-pipelines per-engine streams (normalize chunk c interleaved with
main-phase columns of earlier chunks).  Inverse norms are computed as
Exp(-0.5*Ln(ss)); with the ACT table set pinned to
natural_log_exp_and_others, the kernel performs zero table reloads.
A short warm-up burst of matmuls lifts the PE HAM clock-gate to 8/8
before the first real matmul.

Host-side work is layout-only (concat/transpose/roll/replicate): all
arithmetic (normalization, matmul, softmax, log, reductions) is on
device.
"""

import numpy as np
from contextlib import ExitStack

import concourse.bass as bass
import concourse.tile as tile
from concourse import bacc, mybir
from concourse.bass_utils import run_bass_kernel_spmd
from concourse._compat import with_exitstack

B = 4096
D = 256
N = 2 * B                 # 8192 rows/cols of the similarity matrix
N_CORES = 8
LOCAL = N // N_CORES      # 1024 rows per core
CHUNK = 2048              # normalize / matmul-group column chunk
NCHUNK = N // CHUNK       # 4
KC = D // 128             # 2 contraction chunks of 128
MTILES = LOCAL // 128     # 8 m-tiles of 128 rows
F32 = mybir.dt.float32
BF16 = mybir.dt.bfloat16
AF = mybir.ActivationFunctionType
X = mybir.AxisListType.X

@with_exitstack
def _ntxent_kernel(ctx: ExitStack, tc: tile.TileContext, rt_ap, out_ap):
    nc = tc.nc

    sb_rt = ctx.enter_context(tc.tile_pool(name="rt", bufs=KC * NCHUNK))
    sb_sq = ctx.enter_context(tc.tile_pool(name="sq", bufs=4))
    sb_inv = ctx.enter_context(tc.tile_pool(name="inv", bufs=4))
    sb_znt = ctx.enter_context(tc.tile_pool(name="znt", bufs=1))
    sb_dmy = ctx.enter_context(tc.tile_pool(name="dmy", bufs=2))
    sb_fin = ctx.enter_context(tc.tile_pool(name="fin", bufs=1))
    ps = ctx.enter_context(tc.tile_pool(name="ps", bufs=2, space="PSUM"))

    # constants
    ones128 = sb_fin.tile([128, 1], F32, tag="ones128")
    nc.vector.memset(ones128[:], 1.0)
    onesb = sb_fin.tile([128, 128], BF16, tag="onesb")
    nc.vector.memset(onesb[:], 1.0)
    neg2 = sb_fin.tile([128, 1], F32, tag="neg2")
    nc.vector.memset(neg2[:], -2.0)
    neg1 = sb_fin.tile([128, 1], F32, tag="neg1")
    nc.vector.memset(neg1[:], -1.0)

    # persistent tiles
    znt = sb_znt.tile([128, KC, N], BF16)            # normalized, transposed reps
    racc = sb_fin.tile([128, MTILES * NCHUNK], F32, tag="racc")
    lgacc = sb_fin.tile([128, MTILES], F32, tag="lgacc")
    pacc = sb_fin.tile([128, KC], F32, tag="pacc")

    # all input DMAs issued up-front
    rtk = {}
    for c in range(NCHUNK):
        for k in range(KC):
            t = sb_rt.tile([128, CHUNK], F32, tag="rt")
            nc.sync.dma_start(out=t[:], in_=rt_ap[k][:, bass.ds(c * CHUNK, CHUNK)])
            rtk[(c, k)] = t

    def squares(c):
        sqs = []
        for k in range(KC):
            sq = sb_sq.tile([128, CHUNK], BF16, tag="sq")
            nc.vector.tensor_mul(sq[:], rtk[(c, k)][:], rtk[(c, k)][:])
            sqs.append(sq)
        return sqs

    def colsum(c, sqs):
        # column sums of squares, broadcast across partitions via the
        # ones[128,128] stationary operand
        ns2b = ps.tile([128, CHUNK], F32, tag="ps")
        for k in range(KC):
            for j in range(CHUNK // 512):
                jsl = bass.ds(j * 512, 512)
                nc.tensor.matmul(ns2b[:, jsl], onesb[:], sqs[k][:, jsl],
                                 start=(k == 0), stop=(k == KC - 1))
        return ns2b

    def inv_ln(c, ns2b):
        # first half of 1/sqrt(ss) = exp(-0.5*ln(ss)); both functions
        # live in the pinned natural_log_exp table set (no reloads)
        lnb = sb_inv.tile([128, CHUNK], F32, tag="inv")
        nc.scalar.activation(lnb[:], ns2b[:], AF.Ln, bias=0.0, scale=1.0)
        return lnb

    def inv_exp(c, lnb):
        invb = sb_inv.tile([128, CHUNK], F32, tag="inv")
        nc.scalar.activation(invb[:], lnb[:], AF.Exp, bias=0.0, scale=-0.5)
        return invb

    def apply(c, invb):
        csl = bass.ds(c * CHUNK, CHUNK)
        for k in range(KC):
            nc.vector.tensor_mul(znt[:, k, csl], rtk[(c, k)][:], invb[:])

    def main_column(g, ms):
        # 128x2048 similarity blocks for m-tiles `ms` against chunk g
        for m in ms:
            msl = bass.ds(m * 128, 128)
            pst = ps.tile([128, CHUNK], F32, tag="ps")
            for k in range(KC):
                for j in range(CHUNK // 512):
                    jsl = bass.ds(j * 512, 512)
                    nc.tensor.matmul(pst[:, jsl], znt[:, k, msl],
                                     znt[:, k, bass.ds(g * CHUNK + j * 512, 512)],
                                     start=(k == 0), stop=(k == KC - 1))
            dmy = sb_dmy.tile([128, CHUNK], BF16, tag="dmy")
            nc.scalar.activation(dmy[:], pst[:], AF.Exp, bias=neg2[:], scale=2.0,
                                 accum_out=racc[:, bass.ds(m * NCHUNK + g, 1)])

    def pos_phase():
        # positive-pair cosines for local rows (cols 0:1024 x 4096:5120)
        for k in range(KC):
            pprod = sb_dmy.tile([128, LOCAL], BF16, tag="dmy")
            nc.vector.tensor_mul(pprod[:], znt[:, k, 0:LOCAL],
                                 znt[:, k, bass.ds(B, LOCAL)])
            nc.vector.reduce_sum(pacc[:, bass.ds(k, 1)], pprod[:], axis=X)

    # Pin the ACT table set to natural_log_exp_and_others (id 6): it
    # contains every function this kernel uses (Ln, Exp, Copy), so
    # walrus's lower_act inserts no further table loads.  Without this,
    # Ln and Exp resolve to two different sets and every invnorm pair
    # costs two ~1.3us reloads.
    nc.scalar.add_instruction(mybir.InstLoadActFuncSet(
        name=nc.get_next_instruction_name(), ins=[], outs=[],
        act_func_set_id=6))

    # PE warm-up: ~6us of throwaway matmuls so the HAM clock-gate is at
    # 8/8 before the first real matmul (the PE starts cold at 1.2 GHz;
    # the window needs ~3.4us of sustained activity).  Reads the first
    # input chunk so it starts as soon as that DMA lands.
    wp = ps.tile([128, 128], F32, tag="ps")
    for w in range(20):
        nc.tensor.matmul(wp[:], onesb[:], onesb[:], start=True, stop=True)

    # Hand-pipelined emission. Engines execute their streams in program
    # order, so each engine's stream must avoid waiting on work that a
    # LATER entry of another engine's stream produces.
    #   DVE: sq0 sq1 app0 sq2 app1 sq3 app2 pos app3
    #   PE : warm cs0 cs1 g0[0:4] cs2 g0[4:] g1[0:4] cs3 g1[4:] g2 g3
    #   ACT: inv0 inv1 e(g0,0:4) inv2 e(g0,4:) e(g1,0:4) inv3 e(g1,4:) ...
    front = list(range(MTILES // 2))
    back = list(range(MTILES // 2, MTILES))
    sq0 = squares(0)
    sq1 = squares(1)
    ns0 = colsum(0, sq0)
    ns1 = colsum(1, sq1)
    ln0 = inv_ln(0, ns0)
    ln1 = inv_ln(1, ns1)
    iv0 = inv_exp(0, ln0)
    iv1 = inv_exp(1, ln1)
    apply(0, iv0)
    apply(1, iv1)
    sq2 = squares(2)
    sq3 = squares(3)
    main_column(0, front)
    ns2 = colsum(2, sq2)
    ln2 = inv_ln(2, ns2)
    main_column(0, back)
    iv2 = inv_exp(2, ln2)
    apply(2, iv2)
    main_column(1, front)
    ns3 = colsum(3, sq3)
    ln3 = inv_ln(3, ns3)
    main_column(1, back)
    iv3 = inv_exp(3, ln3)
    apply(3, iv3)
    pos_phase()
    main_column(2, range(MTILES))
    main_column(3, range(MTILES))

    # rowsums -> ln(rowsum - 1) in one op per phase
    rsall = sb_fin.tile([128, MTILES], F32, tag="rsall")
    nc.vector.reduce_sum(
        rsall[:], racc[:].rearrange("p (m g) -> p m g", g=NCHUNK), axis=X)
    nc.scalar.activation(lgacc[:], rsall[:], AF.Ln, bias=neg1[:], scale=1.0)

    # ---- fold to two scalars ----
    fin = sb_fin.tile([128, 2], F32, tag="fin")
    nc.vector.reduce_sum(fin[:, 0:1], lgacc[:], axis=X)
    nc.vector.reduce_sum(fin[:, 1:2], pacc[:], axis=X)
    psf = ps.tile([1, 2], F32, tag="ps")
    nc.tensor.matmul(psf[:], ones128[:], fin[:], start=True, stop=True)
    ob = sb_fin.tile([1, 2], F32, tag="ob")
    nc.scalar.copy(ob[:], psf[:])
    nc.sync.dma_start(out=out_ap[:, :], in_=ob[:])


_NC_CACHE = None


def _build_program():
    global _NC_CACHE
    if _NC_CACHE is not None:
        return _NC_CACHE
    nc = bacc.Bacc("TRN2", target_bir_lowering=False, debug=False,
                   num_devices=N_CORES)
    rt = nc.dram_tensor("rt", [KC, 128, N], F32, kind="ExternalInput").ap()
    out = nc.dram_tensor("out", [1, 2], F32, kind="ExternalOutput").ap()
    with tile.TileContext(nc) as tc:
        _ntxent_kernel(tc, rt, out)
    nc.finalize()
    _NC_CACHE = nc
    return nc


def kernel(zis: np.ndarray, zjs: np.ndarray) -> np.ndarray:
    assert zis.shape == (B, D) and zjs.shape == (B, D)
    nc = _build_program()

    # Host prep (layout only): stack, transpose to [D, N], split the
    # contraction dim, and roll columns so each core's local block is
    # at a uniform offset.
    rt_full = np.ascontiguousarray(
        np.concatenate([zjs, zis], axis=0).T.astype(np.float32, copy=False)
    ).reshape(KC, 128, N)

    in_maps = []
    for c in range(N_CORES):
        rolled = np.roll(rt_full, -c * LOCAL, axis=2)
        in_maps.append({"rt": np.ascontiguousarray(rolled)})

    res = run_bass_kernel_spmd(nc, in_maps, core_ids=list(range(N_CORES)))

    log_sum = 0.0
    pos_sum = 0.0
    for c in range(N_CORES):
        o = res.results[c]["out"]
        log_sum += float(o[0, 0])
        pos_sum += float(o[0, 1])
    loss = 2.0 + (log_sum - 2.0 * pos_sum) / N
    return np.asarray(loss, dtype=np.float32)


# revision 23
# speedup vs baseline: 1.0654x; 1.0508x over previous
"""NT-Xent loss on 8 Trainium2 NeuronCores (Bass/Tile).

Math
----
reference: rows = interleave(zjs, zis) [2B, D]; zn = rows/max(|row|,eps);
S = (zn @ zn.T)/0.5; mask diag; loss = -mean_i log_softmax(S)[i, pair(i)].

The loss is invariant to any joint row/column permutation, so we use the
STACKED order rows = [zjs; zis] with pair(i) = i +- B.  Since every score
is 2*cos <= 2 and the diagonal exp(2*cos_ii - 2) == 1 (+/- fp eps), no
masking or row-max pass is needed:

    lse_i  = 2 + ln( sum_j exp(2 cos_ij - 2) - 1 )
    loss   = 2 + ( sum_i ln(rowsum_i - 1) - 2 * sum_i cos_{i,pair(i)} ) / 2B

Distribution: each core gets the full transposed rep matrix [D, 2B]
(= "all-gathered Zn" state), ROLLED along columns by c*1024 so that the
uniform SPMD program always treats columns [0:1024] as its local row
block and [4096:5120] as the positive partners.  Each core normalizes
the full matrix (cheap), computes its 1024x8192 block of the similarity
matrix in bf16 on the TensorEngine, fuses exp+rowsum on the Scalar
engine (accum_out), and emits two partial sums; the host combines 8
pairs of scalars.

Scheduling: engines execute instructions in program order, so emission
hand-pipelines per-engine streams (normalize chunk c interleaved with
main-phase columns of earlier chunks).  Inverse norms are computed as
Exp(-0.5*Ln(ss)); with the ACT table set pinned to
natural_log_exp_and_others, the kernel performs zero table reloads.
A short warm-up burst of matmuls lifts the PE HAM clock-gate to 8/8
before the first real matmul.

Host-side work is layout-only (concat/transpose/roll/replicate): all
arithmetic (normalization, matmul, softmax, log, reductions) is on
device.
"""

import numpy as np
from contextlib import ExitStack

import concourse.bass as bass
import concourse.tile as tile
from concourse import bacc, mybir
from concourse.bass_utils import run_bass_kernel_spmd
from concourse._compat import with_exitstack

B = 4096
D = 256
N = 2 * B                 # 8192 rows/cols of the similarity matrix
N_CORES = 8
LOCAL = N // N_CORES      # 1024 rows per core
CHUNK = 2048              # normalize / matmul-group column chunk
NCHUNK = N // CHUNK       # 4
KC = D // 128             # 2 contraction chunks of 128
MTILES = LOCAL // 128     # 8 m-tiles of 128 rows
F32 = mybir.dt.float32
BF16 = mybir.dt.bfloat16
AF = mybir.ActivationFunctionType
X = mybir.AxisListType.X

@with_exitstack
def _ntxent_kernel(ctx: ExitStack, tc: tile.TileContext, rt_ap, out_ap):
    nc = tc.nc

    sb_rt = ctx.enter_context(tc.tile_pool(name="rt", bufs=2 * KC * NCHUNK))
    sb_sq = ctx.enter_context(tc.tile_pool(name="sq", bufs=4))
    sb_inv = ctx.enter_context(tc.tile_pool(name="inv", bufs=2))
    sb_znt = ctx.enter_context(tc.tile_pool(name="znt", bufs=1))
    sb_dmy = ctx.enter_context(tc.tile_pool(name="dmy", bufs=2))
    sb_fin = ctx.enter_context(tc.tile_pool(name="fin", bufs=1))
    ps = ctx.enter_context(tc.tile_pool(name="ps", bufs=2, space="PSUM"))

    # constants
    ones128 = sb_fin.tile([128, 1], F32, tag="ones128")
    nc.vector.memset(ones128[:], 1.0)
    onesb = sb_fin.tile([128, 128], BF16, tag="onesb")
    nc.vector.memset(onesb[:], 1.0)
    neg2 = sb_fin.tile([128, 1], F32, tag="neg2")
    nc.vector.memset(neg2[:], -2.0)
    neg1 = sb_fin.tile([128, 1], F32, tag="neg1")
    nc.vector.memset(neg1[:], -1.0)

    # persistent tiles
    znt = sb_znt.tile([128, KC, N], BF16)            # normalized, transposed reps
    racc = sb_fin.tile([128, MTILES * NCHUNK], F32, tag="racc")
    lgacc = sb_fin.tile([128, MTILES], F32, tag="lgacc")
    pacc = sb_fin.tile([128, KC], F32, tag="pacc")

    # all input DMAs issued up-front
    rtk = {}
    for c in range(NCHUNK):
        for k in range(KC):
            t = sb_rt.tile([128, CHUNK], F32, tag="rt")
            nc.sync.dma_start(out=t[:], in_=rt_ap[k][:, bass.ds(c * CHUNK, CHUNK)])
            rtk[(c, k)] = t

    def squares(c):
        sqs = []
        for k in range(KC):
            sq = sb_sq.tile([128, CHUNK], BF16, tag="sq")
            nc.vector.tensor_mul(sq[:], rtk[(c, k)][:], rtk[(c, k)][:])
            sqs.append(sq)
        return sqs

    def colsum(c, sqs):
        # column sums of squares, broadcast across partitions via the
        # ones[128,128] stationary operand
        ns2b = ps.tile([128, CHUNK], F32, tag="ps")
        for k in range(KC):
            for j in range(CHUNK // 512):
                jsl = bass.ds(j * 512, 512)
                nc.tensor.matmul(ns2b[:, jsl], onesb[:], sqs[k][:, jsl],
                                 start=(k == 0), stop=(k == KC - 1))
        return ns2b

    def invnorm(c, ns2b):
        # 1/sqrt(ss) = exp(-0.5*ln(ss)); both functions live in the
        # pinned natural_log_exp table set, so no table reloads.
        lnb = sb_inv.tile([128, CHUNK], F32, tag="inv")
        nc.scalar.activation(lnb[:], ns2b[:], AF.Ln, bias=0.0, scale=1.0)
        invb = sb_inv.tile([128, CHUNK], F32, tag="inv")
        nc.scalar.activation(invb[:], lnb[:], AF.Exp, bias=0.0, scale=-0.5)
        return invb

    def apply(c, invb):
        csl = bass.ds(c * CHUNK, CHUNK)
        for k in range(KC):
            nc.vector.tensor_mul(znt[:, k, csl], rtk[(c, k)][:], invb[:])

    def main_column(g, ms):
        # 128x2048 similarity blocks for m-tiles `ms` against chunk g
        for m in ms:
            msl = bass.ds(m * 128, 128)
            pst = ps.tile([128, CHUNK], F32, tag="ps")
            for k in range(KC):
                for j in range(CHUNK // 512):
                    jsl = bass.ds(j * 512, 512)
                    nc.tensor.matmul(pst[:, jsl], znt[:, k, msl],
                                     znt[:, k, bass.ds(g * CHUNK + j * 512, 512)],
                                     start=(k == 0), stop=(k == KC - 1))
            dmy = sb_dmy.tile([128, CHUNK], BF16, tag="dmy")
            nc.scalar.activation(dmy[:], pst[:], AF.Exp, bias=neg2[:], scale=2.0,
                                 accum_out=racc[:, bass.ds(m * NCHUNK + g, 1)])

    def pos_phase():
        # positive-pair cosines for local rows (cols 0:1024 x 4096:5120)
        for k in range(KC):
            pprod = sb_dmy.tile([128, LOCAL], BF16, tag="dmy")
            nc.vector.tensor_mul(pprod[:], znt[:, k, 0:LOCAL],
                                 znt[:, k, bass.ds(B, LOCAL)])
            nc.vector.reduce_sum(pacc[:, bass.ds(k, 1)], pprod[:], axis=X)

    # Pin the ACT table set to natural_log_exp_and_others (id 6): it
    # contains every function this kernel uses (Ln, Exp, Copy), so
    # walrus's lower_act inserts no further table loads.  Without this,
    # Ln and Exp resolve to two different sets and every invnorm pair
    # costs two ~1.3us reloads.
    nc.scalar.add_instruction(mybir.InstLoadActFuncSet(
        name=nc.get_next_instruction_name(), ins=[], outs=[],
        act_func_set_id=6))

    # PE warm-up: ~6us of throwaway matmuls so the HAM clock-gate is at
    # 8/8 before the first real matmul (the PE starts cold at 1.2 GHz;
    # the window needs ~3.4us of sustained activity).
    wrm = sb_fin.tile([128, 512], BF16, tag="wrm")
    nc.gpsimd.memset(wrm[:], 0.0)
    wp = ps.tile([128, 512], F32, tag="ps")
    for _ in range(14):
        nc.tensor.matmul(wp[:], onesb[:], wrm[:], start=True, stop=True)

    # Hand-pipelined emission. Engines execute their streams in program
    # order, so each engine's stream must avoid waiting on work that a
    # LATER entry of another engine's stream produces.
    #   DVE: sq0 sq1 app0 sq2 app1 sq3 app2 pos app3
    #   PE : warm cs0 cs1 g0[0:4] cs2 g0[4:] g1[0:4] cs3 g1[4:] g2 g3
    #   ACT: inv0 inv1 e(g0,0:4) inv2 e(g0,4:) e(g1,0:4) inv3 e(g1,4:) ...
    front = list(range(MTILES // 2))
    back = list(range(MTILES // 2, MTILES))
    sq0 = squares(0)
    sq1 = squares(1)
    ns0 = colsum(0, sq0)
    ns1 = colsum(1, sq1)
    iv0 = invnorm(0, ns0)
    apply(0, iv0)
    sq2 = squares(2)
    iv1 = invnorm(1, ns1)
    apply(1, iv1)
    main_column(0, front)
    ns2 = colsum(2, sq2)
    iv2 = invnorm(2, ns2)
    sq3 = squares(3)
    apply(2, iv2)
    main_column(0, back)
    main_column(1, front)
    ns3 = colsum(3, sq3)
    iv3 = invnorm(3, ns3)
    apply(3, iv3)
    pos_phase()
    main_column(1, back)
    main_column(2, range(MTILES))
    main_column(3, range(MTILES))

    # rowsums -> ln(rowsum - 1) in one op per phase
    rsall = sb_fin.tile([128, MTILES], F32, tag="rsall")
    nc.vector.reduce_sum(
        rsall[:], racc[:].rearrange("p (m g) -> p m g", g=NCHUNK), axis=X)
    nc.scalar.activation(lgacc[:], rsall[:], AF.Ln, bias=neg1[:], scale=1.0)

    # ---- fold to two scalars ----
    fin = sb_fin.tile([128, 2], F32, tag="fin")
    nc.vector.reduce_sum(fin[:, 0:1], lgacc[:], axis=X)
    nc.vector.reduce_sum(fin[:, 1:2], pacc[:], axis=X)
    psf = ps.tile([1, 2], F32, tag="ps")
    nc.tensor.matmul(psf[:], ones128[:], fin[:], start=True, stop=True)
    ob = sb_fin.tile([1, 2], F32, tag="ob")
    nc.scalar.copy(ob[:], psf[:])
    nc.sync.dma_start(out=out_ap[:, :], in_=ob[:])


_NC_CACHE = None


def _build_program():
    global _NC_CACHE
    if _NC_CACHE is not None:
        return _NC_CACHE
    nc = bacc.Bacc("TRN2", target_bir_lowering=False, debug=False,
                   num_devices=N_CORES)
    rt = nc.dram_tensor("rt", [KC, 128, N], F32, kind="ExternalInput").ap()
    out = nc.dram_tensor("out", [1, 2], F32, kind="ExternalOutput").ap()
    with tile.TileContext(nc) as tc:
        _ntxent_kernel(tc, rt, out)
    nc.finalize()
    _NC_CACHE = nc
    return nc


def kernel(zis: np.ndarray, zjs: np.ndarray) -> np.ndarray:
    assert zis.shape == (B, D) and zjs.shape == (B, D)
    nc = _build_program()

    # Host prep (layout only): stack, transpose to [D, N], split the
    # contraction dim, and roll columns so each core's local block is
    # at a uniform offset.
    rt_full = np.ascontiguousarray(
        np.concatenate([zjs, zis], axis=0).T.astype(np.float32, copy=False)
    ).reshape(KC, 128, N)

    in_maps = []
    for c in range(N_CORES):
        rolled = np.roll(rt_full, -c * LOCAL, axis=2)
        in_maps.append({"rt": np.ascontiguousarray(rolled)})

    res = run_bass_kernel_spmd(nc, in_maps, core_ids=list(range(N_CORES)))

    log_sum = 0.0
    pos_sum = 0.0
    for c in range(N_CORES):
        o = res.results[c]["out"]
        log_sum += float(o[0, 0])
        pos_sum += float(o[0, 1])
    loss = 2.0 + (log_sum - 2.0 * pos_sum) / N
    return np.asarray(loss, dtype=np.float32)


# revision 24
# speedup vs baseline: 1.0691x; 1.0035x over previous
"""NT-Xent loss on 8 Trainium2 NeuronCores (Bass/Tile).

Math
----
reference: rows = interleave(zjs, zis) [2B, D]; zn = rows/max(|row|,eps);
S = (zn @ zn.T)/0.5; mask diag; loss = -mean_i log_softmax(S)[i, pair(i)].

The loss is invariant to any joint row/column permutation, so we use the
STACKED order rows = [zjs; zis] with pair(i) = i +- B.  Since every score
is 2*cos <= 2 and the diagonal exp(2*cos_ii - 2) == 1 (+/- fp eps), no
masking or row-max pass is needed:

    lse_i  = 2 + ln( sum_j exp(2 cos_ij - 2) - 1 )
    loss   = 2 + ( sum_i ln(rowsum_i - 1) - 2 * sum_i cos_{i,pair(i)} ) / 2B

Distribution: each core gets the full transposed rep matrix [D, 2B]
(= "all-gathered Zn" state), ROLLED along columns by c*1024 so that the
uniform SPMD program always treats columns [0:1024] as its local row
block and [4096:5120] as the positive partners.  Each core normalizes
the full matrix (cheap), computes its 1024x8192 block of the similarity
matrix in bf16 on the TensorEngine, fuses exp+rowsum on the Scalar
engine (accum_out), and emits two partial sums; the host combines 8
pairs of scalars.

Scheduling: engines execute instructions in program order, so emission
hand-pipelines per-engine streams (normalize chunk c interleaved with
main-phase columns of earlier chunks).  Inverse norms are computed as
Exp(-0.5*Ln(ss)); with the ACT table set pinned to
natural_log_exp_and_others, the kernel performs zero table reloads.
A short warm-up burst of matmuls lifts the PE HAM clock-gate to 8/8
before the first real matmul.

Host-side work is layout-only (concat/transpose/roll/replicate): all
arithmetic (normalization, matmul, softmax, log, reductions) is on
device.
"""

import numpy as np
from contextlib import ExitStack

import concourse.bass as bass
import concourse.tile as tile
from concourse import bacc, mybir
from concourse.bass_utils import run_bass_kernel_spmd
from concourse._compat import with_exitstack

B = 4096
D = 256
N = 2 * B                 # 8192 rows/cols of the similarity matrix
N_CORES = 8
LOCAL = N // N_CORES      # 1024 rows per core
CHUNK = 2048              # normalize / matmul-group column chunk
NCHUNK = N // CHUNK       # 4
KC = D // 128             # 2 contraction chunks of 128
MTILES = LOCAL // 128     # 8 m-tiles of 128 rows
F32 = mybir.dt.float32
BF16 = mybir.dt.bfloat16
FP8 = mybir.dt.float8e4
AF = mybir.ActivationFunctionType
X = mybir.AxisListType.X

@with_exitstack
def _ntxent_kernel(ctx: ExitStack, tc: tile.TileContext, rt_ap, out_ap):
    nc = tc.nc

    sb_rt = ctx.enter_context(tc.tile_pool(name="rt", bufs=2 * KC * NCHUNK))
    sb_sq = ctx.enter_context(tc.tile_pool(name="sq", bufs=4))
    sb_inv = ctx.enter_context(tc.tile_pool(name="inv", bufs=2))
    sb_znt = ctx.enter_context(tc.tile_pool(name="znt", bufs=1))
    sb_dmy = ctx.enter_context(tc.tile_pool(name="dmy", bufs=2))
    sb_fin = ctx.enter_context(tc.tile_pool(name="fin", bufs=1))
    ps = ctx.enter_context(tc.tile_pool(name="ps", bufs=2, space="PSUM"))

    # constants
    ones128 = sb_fin.tile([128, 1], F32, tag="ones128")
    nc.vector.memset(ones128[:], 1.0)
    onesb = sb_fin.tile([128, 128], BF16, tag="onesb")
    nc.vector.memset(onesb[:], 1.0)
    neg2 = sb_fin.tile([128, 1], F32, tag="neg2")
    nc.vector.memset(neg2[:], -2.0)
    neg1 = sb_fin.tile([128, 1], F32, tag="neg1")
    nc.vector.memset(neg1[:], -1.0)

    # persistent tiles
    znt = sb_znt.tile([128, KC, N], FP8)             # normalized, transposed reps (fp8 for DoubleRow)
    racc = sb_fin.tile([128, MTILES * NCHUNK], F32, tag="racc")
    lgacc = sb_fin.tile([128, MTILES], F32, tag="lgacc")
    pacc = sb_fin.tile([128, KC], F32, tag="pacc")

    # all input DMAs issued up-front
    rtk = {}
    for c in range(NCHUNK):
        for k in range(KC):
            t = sb_rt.tile([128, CHUNK], F32, tag="rt")
            nc.sync.dma_start(out=t[:], in_=rt_ap[k][:, bass.ds(c * CHUNK, CHUNK)])
            rtk[(c, k)] = t

    def squares(c):
        sqs = []
        for k in range(KC):
            sq = sb_sq.tile([128, CHUNK], BF16, tag="sq")
            nc.vector.tensor_mul(sq[:], rtk[(c, k)][:], rtk[(c, k)][:])
            sqs.append(sq)
        return sqs

    def colsum(c, sqs):
        # column sums of squares, broadcast across partitions via the
        # ones[128,128] stationary operand
        ns2b = ps.tile([128, CHUNK], F32, tag="ps")
        for k in range(KC):
            for j in range(CHUNK // 512):
                jsl = bass.ds(j * 512, 512)
                nc.tensor.matmul(ns2b[:, jsl], onesb[:], sqs[k][:, jsl],
                                 start=(k == 0), stop=(k == KC - 1))
        return ns2b

    def invnorm(c, ns2b):
        # 1/sqrt(ss) = exp(-0.5*ln(ss)); both functions live in the
        # pinned natural_log_exp table set, so no table reloads.
        lnb = sb_inv.tile([128, CHUNK], F32, tag="inv")
        nc.scalar.activation(lnb[:], ns2b[:], AF.Ln, bias=0.0, scale=1.0)
        invb = sb_inv.tile([128, CHUNK], F32, tag="inv")
        nc.scalar.activation(invb[:], lnb[:], AF.Exp, bias=0.0, scale=-0.5)
        return invb

    def apply(c, invb):
        csl = bass.ds(c * CHUNK, CHUNK)
        for k in range(KC):
            nc.vector.tensor_mul(znt[:, k, csl], rtk[(c, k)][:], invb[:])

    def main_column(g, ms):
        # 128x2048 similarity blocks for m-tiles `ms` against chunk g
        for m in ms:
            msl = bass.ds(m * 128, 128)
            pst = ps.tile([128, CHUNK], F32, tag="ps")
            for j in range(CHUNK // 512):
                jsl = bass.ds(j * 512, 512)
                nc.tensor.matmul(pst[:, jsl], znt[:, :, msl],
                                 znt[:, :, bass.ds(g * CHUNK + j * 512, 512)],
                                 start=True, stop=True,
                                 perf_mode=mybir.MatmulPerfMode.DoubleRow)
            dmy = sb_dmy.tile([128, CHUNK], BF16, tag="dmy")
            nc.scalar.activation(dmy[:], pst[:], AF.Exp, bias=neg2[:], scale=2.0,
                                 accum_out=racc[:, bass.ds(m * NCHUNK + g, 1)])

    def pos_phase():
        # positive-pair cosines for local rows (cols 0:1024 x 4096:5120)
        for k in range(KC):
            pprod = sb_dmy.tile([128, LOCAL], BF16, tag="dmy")
            nc.vector.tensor_mul(pprod[:], znt[:, k, 0:LOCAL],
                                 znt[:, k, bass.ds(B, LOCAL)])
            nc.vector.reduce_sum(pacc[:, bass.ds(k, 1)], pprod[:], axis=X)

    # Pin the ACT table set to natural_log_exp_and_others (id 6): it
    # contains every function this kernel uses (Ln, Exp, Copy), so
    # walrus's lower_act inserts no further table loads.  Without this,
    # Ln and Exp resolve to two different sets and every invnorm pair
    # costs two ~1.3us reloads.
    nc.scalar.add_instruction(mybir.InstLoadActFuncSet(
        name=nc.get_next_instruction_name(), ins=[], outs=[],
        act_func_set_id=6))

    # PE warm-up: ~6us of throwaway matmuls so the HAM clock-gate is at
    # 8/8 before the first real matmul (the PE starts cold at 1.2 GHz;
    # the window needs ~3.4us of sustained activity).
    wrm = sb_fin.tile([128, 512], BF16, tag="wrm")
    nc.gpsimd.memset(wrm[:], 0.0)
    wp = ps.tile([128, 512], F32, tag="ps")
    for _ in range(14):
        nc.tensor.matmul(wp[:], onesb[:], wrm[:], start=True, stop=True)

    # Hand-pipelined emission. Engines execute their streams in program
    # order, so each engine's stream must avoid waiting on work that a
    # LATER entry of another engine's stream produces.
    #   DVE: sq0 sq1 app0 sq2 app1 sq3 app2 pos app3
    #   PE : warm cs0 cs1 g0[0:4] cs2 g0[4:] g1[0:4] cs3 g1[4:] g2 g3
    #   ACT: inv0 inv1 e(g0,0:4) inv2 e(g0,4:) e(g1,0:4) inv3 e(g1,4:) ...
    front = list(range(MTILES // 2))
    back = list(range(MTILES // 2, MTILES))
    sq0 = squares(0)
    sq1 = squares(1)
    ns0 = colsum(0, sq0)
    ns1 = colsum(1, sq1)
    iv0 = invnorm(0, ns0)
    apply(0, iv0)
    sq2 = squares(2)
    iv1 = invnorm(1, ns1)
    apply(1, iv1)
    main_column(0, front)
    ns2 = colsum(2, sq2)
    iv2 = invnorm(2, ns2)
    sq3 = squares(3)
    apply(2, iv2)
    main_column(0, back)
    main_column(1, front)
    ns3 = colsum(3, sq3)
    iv3 = invnorm(3, ns3)
    apply(3, iv3)
    pos_phase()
    main_column(1, back)
    main_column(2, range(MTILES))
    main_column(3, range(MTILES))

    # rowsums -> ln(rowsum - 1) in one op per phase
    rsall = sb_fin.tile([128, MTILES], F32, tag="rsall")
    nc.vector.reduce_sum(
        rsall[:], racc[:].rearrange("p (m g) -> p m g", g=NCHUNK), axis=X)
    nc.scalar.activation(lgacc[:], rsall[:], AF.Ln, bias=neg1[:], scale=1.0)

    # ---- fold to two scalars ----
    fin = sb_fin.tile([128, 2], F32, tag="fin")
    nc.vector.reduce_sum(fin[:, 0:1], lgacc[:], axis=X)
    nc.vector.reduce_sum(fin[:, 1:2], pacc[:], axis=X)
    psf = ps.tile([1, 2], F32, tag="ps")
    nc.tensor.matmul(psf[:], ones128[:], fin[:], start=True, stop=True)
    ob = sb_fin.tile([1, 2], F32, tag="ob")
    nc.scalar.copy(ob[:], psf[:])
    nc.sync.dma_start(out=out_ap[:, :], in_=ob[:])


_NC_CACHE = None


def _build_program():
    global _NC_CACHE
    if _NC_CACHE is not None:
        return _NC_CACHE
    nc = bacc.Bacc("TRN2", target_bir_lowering=False, debug=False,
                   num_devices=N_CORES)
    rt = nc.dram_tensor("rt", [KC, 128, N], F32, kind="ExternalInput").ap()
    out = nc.dram_tensor("out", [1, 2], F32, kind="ExternalOutput").ap()
    with tile.TileContext(nc) as tc:
        _ntxent_kernel(tc, rt, out)
    nc.finalize()
    _NC_CACHE = nc
    return nc


def kernel(zis: np.ndarray, zjs: np.ndarray) -> np.ndarray:
    assert zis.shape == (B, D) and zjs.shape == (B, D)
    nc = _build_program()

    # Host prep (layout only): stack, transpose to [D, N], split the
    # contraction dim, and roll columns so each core's local block is
    # at a uniform offset.
    rt_full = np.ascontiguousarray(
        np.concatenate([zjs, zis], axis=0).T.astype(np.float32, copy=False)
    ).reshape(KC, 128, N)

    in_maps = []
    for c in range(N_CORES):
        rolled = np.roll(rt_full, -c * LOCAL, axis=2)
        in_maps.append({"rt": np.ascontiguousarray(rolled)})

    res = run_bass_kernel_spmd(nc, in_maps, core_ids=list(range(N_CORES)))

    log_sum = 0.0
    pos_sum = 0.0
    for c in range(N_CORES):
        o = res.results[c]["out"]
        log_sum += float(o[0, 0])
        pos_sum += float(o[0, 1])
    loss = 2.0 + (log_sum - 2.0 * pos_sum) / N
    return np.asarray(loss, dtype=np.float32)
